# revision 3
# baseline (speedup 1.0000x reference)
"""Masked multi-head attention kernel for 8 Trainium2 NeuronCores.

Strategy (v2 — PE-packing rework of the v1 baseline):
  - 24 (batch, head) pairs sharded as: core c -> batch c//4, heads [3*(c%4) .. 3*(c%4)+2].
  - Key-padding mask handled by HOST-side gather: only unmasked key positions are
    shipped/computed. Padded key slots get zeroed K columns (scores=0 -> exp=1)
    and a 0 in the indicator slot of V, so they contribute nothing.
  - Softmax without max-subtraction (scores ~ N(0,1); masked keys excluded).
  - Row-sum of exp folded into the AV matmul via an indicator slot on V.
  - PE tile packing (the big v2 win): all D=64-contraction matmuls run as
    concurrent PAIRS in the two 64-row halves of the PE array
    (tile_position auto-derived from partition bases):
      * scores: head0 (partitions 0-63) paired with head1 (64-127) per key
        block; head2 paired with itself via duplicated kT2/qT2 (dup'd by
        SBUF->SBUF DMA, free on the DMA engines).
      * out-proj: the K=64 tail (head2 dims) packed as column-group pairs
        (even cg rows 0-63, odd cg rows 64-127) against duplicated OT2.
      * qT2/kT2 projections: M=64 outputs packed 2 query-chunks per pass
        via column tiling.
  - One exp per score pair: [128, 2*512] ACTIVATE (amortizes ACT overhead,
    covers both heads of the pair).
  - ACT table preloaded by a dummy exp at t=0; all DMAs issued on the
    Sync + GpSimd queues so the Scalar engine does nothing but exp.
  - V indicator slots built on-device (gpsimd memset) + a tiny DMA for the
    padded tail block.
  - PSUM: 4 banks score pairs (x2), 2 banks AV accumulators, 2 banks proj.
  - bf16 matmul inputs, fp32 PSUM accumulation, bf16 output partials
    (host sums the 4 partials per batch in fp32, adds proj_b).
"""

import math

import numpy as np
import ml_dtypes

BF16 = ml_dtypes.bfloat16
B, N, C = 2, 2048, 768
H = 12
D = 64
HPC = 3          # heads per core
P = 128
QB = 512         # query block
NQB = N // QB
SCALE = D ** -0.5
NCORES = 8


def _chunks(total, size=QB):
    return [(o, min(size, total - o)) for o in range(0, total, size)]


def _build_program(KP: int):
    from concourse import bacc, mybir
    from concourse.tile import TileContext

    JG = KP // P
    f32 = mybir.dt.float32
    bf16 = mybir.dt.bfloat16
    Exp = mybir.ActivationFunctionType.Exp
    nc = bacc.Bacc(None, target_bir_lowering=False)

    xT_d = nc.declare_dram_parameter("xT", [P, 6, N], bf16, False)
    xTk_d = nc.declare_dram_parameter("xTk", [P, 6, KP], bf16, False)
    kfL_d = nc.declare_dram_parameter("kfL", [P, HPC, D], bf16, False)
    wq_d = nc.declare_dram_parameter("wqT", [P, 6, 192], bf16, False)
    wk_d = nc.declare_dram_parameter("wkT", [P, 6, 192], bf16, False)
    wv_d = nc.declare_dram_parameter("wvT", [P, 6, 192], bf16, False)
    pT01_d = nc.declare_dram_parameter("pT01", [P, 6, P], bf16, False)
    pT2_d = nc.declare_dram_parameter("pT2pk", [P, 3, P], bf16, False)
    out_d = nc.declare_dram_parameter("outT", [P, NQB, 6, QB], bf16, True)

    NHALF = N // 2

    with TileContext(nc) as tc:
        with (
            tc.tile_pool(name="const", bufs=1) as cpool,
            tc.tile_pool(name="work", bufs=1) as wpool,
            tc.tile_pool(name="pt", bufs=4) as ptpool,
            tc.tile_pool(name="rb", bufs=3) as rbpool,
            tc.tile_pool(name="outp", bufs=2) as opool,
            tc.tile_pool(name="ps", bufs=2, space="PSUM") as pspool,
            tc.tile_pool(name="po", bufs=2, space="PSUM") as popool,
            tc.tile_pool(name="pp", bufs=1, space="PSUM") as pppool,
        ):
            # ---- ACT exp-table preload: dummy exp with no data deps so the
            # ~2.7us ACT_TABLE_LOAD runs during the DMA phase.
            warm_i = cpool.tile([1, 8], f32)
            warm_o = cpool.tile([1, 8], bf16)
            nc.vector.memset(warm_i[:], 0.0)
            nc.scalar.activation(warm_o[:], warm_i[:], Exp)

            # ---- constant tiles + input DMAs.
            # Scalar queue: small weight tensors only (done before first exp).
            # Sync/GpSimd queues: bulk x / xk traffic, halves split across
            # the two queues.
            wq = cpool.tile([P, 6, 192], bf16)
            wk = cpool.tile([P, 6, 192], bf16)
            wv = cpool.tile([P, 6, 192], bf16)
            xT = cpool.tile([P, 6, N], bf16)
            xTk = cpool.tile([P, 6, KP], bf16)
            pT01 = cpool.tile([P, 6, P], bf16)
            pT2 = cpool.tile([P, 3, P], bf16)

            nc.scalar.dma_start(wk[:], wk_d[:])
            nc.scalar.dma_start(wq[:], wq_d[:])
            nc.scalar.dma_start(wv[:], wv_d[:])
            nc.scalar.dma_start(pT01[:], pT01_d[:])
            nc.scalar.dma_start(pT2[:], pT2_d[:])
            for t in range(6):
                q = nc.sync if t < 3 else nc.gpsimd
                q.dma_start(xTk[:, t : t + 1, :], xTk_d[:, t : t + 1, :])
            for t in range(6):
                nc.sync.dma_start(
                    xT[:, t : t + 1, 0:NHALF], xT_d[:, t : t + 1, 0:NHALF]
                )
                nc.gpsimd.dma_start(
                    xT[:, t : t + 1, NHALF:N], xT_d[:, t : t + 1, NHALF:N]
                )

            # ---- work tiles
            qT01 = wpool.tile([P, N], bf16)
            qT2d = wpool.tile([P, N], bf16)   # head2 q, duplicated in halves
            kT01 = wpool.tile([P, KP], bf16)
            kT2d = wpool.tile([P, KP], bf16)  # head2 k, duplicated in halves
            v_sb = wpool.tile([P, JG, 2 * HPC, D], bf16)
            OT01 = wpool.tile([P, NQB, QB], bf16)
            OT2d = wpool.tile([P, NQB, QB], bf16)  # head2 O^T, duplicated

            # V indicator slots: 1.0 everywhere except the padded tail block,
            # which comes from a tiny DMA.
            if JG > 1:
                nc.gpsimd.memset(v_sb[:, 0 : JG - 1, 0 : 2 * HPC : 2, :], 1.0)
            nc.scalar.dma_start(v_sb[:, JG - 1, 0 : 2 * HPC : 2, :], kfL_d[:])

            # ---- k projections.
            # kT01 (heads 0,1; M=128): two 512-col chunks packed per PSUM tile.
            kchunks = _chunks(KP)
            for i in range(0, len(kchunks), 2):
                pair = kchunks[i : i + 2]
                ps = pspool.tile([P, 2, QB], f32, name="psq", tag="ps")
                for t in range(6):
                    for s, (o, sz) in enumerate(pair):
                        nc.tensor.matmul(
                            ps[:, s, 0:sz],
                            wk[:, t, 0:P],
                            xTk[:, t, o : o + sz],
                            start=(t == 0),
                            stop=(t == 5),
                        )
                if len(pair) == 2 and pair[0][1] == QB and pair[1][1] == QB:
                    nc.vector.tensor_copy(
                        kT01[:, pair[0][0] : pair[0][0] + 2 * QB], ps[:, :, :]
                    )
                else:
                    for s, (o, sz) in enumerate(pair):
                        nc.vector.tensor_copy(kT01[:, o : o + sz], ps[:, s, 0:sz])

            # kT2 (head 2; M=64): chunk pairs packed via column tiling, then
            # duplicated into partitions 64-127 by SBUF->SBUF DMA.
            for i in range(0, len(kchunks), 2):
                pair = kchunks[i : i + 2]
                ps = pspool.tile([P, 2, QB], f32, name="psq2", tag="ps")
                for t in range(6):
                    (o0, sz0) = pair[0]
                    nc.tensor.matmul(
                        ps[0:D, 0, 0:sz0],
                        wk[:, t, P:192],
                        xTk[:, t, o0 : o0 + sz0],
                        start=(t == 0),
                        stop=(t == 5),
                    )
                    if len(pair) == 2:
                        (o1, sz1) = pair[1]
                        nc.tensor.matmul(
                            ps[D:P, 1, 0:sz1],
                            wk[:, t, P:192],
                            xTk[:, t, o1 : o1 + sz1],
                            start=(t == 0),
                            stop=(t == 5),
                        )
                nc.vector.tensor_copy(
                    kT2d[0:D, pair[0][0] : pair[0][0] + pair[0][1]],
                    ps[0:D, 0, 0 : pair[0][1]],
                )
                if len(pair) == 2:
                    nc.vector.tensor_copy(
                        kT2d[0:D, pair[1][0] : pair[1][0] + pair[1][1]],
                        ps[D:P, 1, 0 : pair[1][1]],
                    )
            nc.sync.dma_start(kT2d[D:P, :], kT2d[0:D, :])

            # ---- q projections for a range of 512-query chunks.
            def q_proj(c0, c1):
                qchunks = [(o, QB) for o in range(c0 * QB, c1 * QB, QB)]
                for i in range(0, len(qchunks), 2):
                    pair = qchunks[i : i + 2]
                    ps = pspool.tile([P, 2, QB], f32, name="psq", tag="ps")
                    for t in range(6):
                        for s, (o, _) in enumerate(pair):
                            nc.tensor.matmul(
                                ps[:, s, :],
                                wq[:, t, 0:P],
                                xT[:, t, o : o + QB],
                                start=(t == 0),
                                stop=(t == 5),
                            )
                    if len(pair) == 2:
                        nc.vector.tensor_copy(
                            qT01[:, pair[0][0] : pair[0][0] + 2 * QB], ps[:, :, :]
                        )
                    else:
                        nc.vector.tensor_copy(
                            qT01[:, pair[0][0] : pair[0][0] + QB], ps[:, 0, :]
                        )
                for i in range(0, len(qchunks), 2):
                    pair = qchunks[i : i + 2]
                    ps = pspool.tile([P, 2, QB], f32, name="psq2", tag="ps")
                    if len(pair) == 2:
                        # two chunks packed in column halves, dup'd by DMA
                        for t in range(6):
                            nc.tensor.matmul(
                                ps[0:D, 0, :],
                                wq[:, t, P:192],
                                xT[:, t, pair[0][0] : pair[0][0] + QB],
                                start=(t == 0),
                                stop=(t == 5),
                            )
                            nc.tensor.matmul(
                                ps[D:P, 1, :],
                                wq[:, t, P:192],
                                xT[:, t, pair[1][0] : pair[1][0] + QB],
                                start=(t == 0),
                                stop=(t == 5),
                            )
                        nc.vector.tensor_copy(
                            qT2d[0:D, pair[0][0] : pair[0][0] + QB], ps[0:D, 0, :]
                        )
                        nc.vector.tensor_copy(
                            qT2d[0:D, pair[1][0] : pair[1][0] + QB], ps[D:P, 1, :]
                        )
                        nc.sync.dma_start(
                            qT2d[D:P, pair[0][0] : pair[1][0] + QB],
                            qT2d[0:D, pair[0][0] : pair[1][0] + QB],
                        )
                    else:
                        # single chunk: same chunk in both column halves ->
                        # qT2d comes out duplicated with no DMA
                        o = pair[0][0]
                        for t in range(6):
                            nc.tensor.matmul(
                                ps[0:D, 0, :],
                                wq[:, t, P:192],
                                xT[:, t, o : o + QB],
                                start=(t == 0),
                                stop=(t == 5),
                            )
                            nc.tensor.matmul(
                                ps[D:P, 1, :],
                                wq[:, t, P:192],
                                xT[:, t, o : o + QB],
                                start=(t == 0),
                                stop=(t == 5),
                            )
                        nc.vector.tensor_copy(qT2d[0:D, o : o + QB], ps[0:D, 0, :])
                        nc.vector.tensor_copy(qT2d[D:P, o : o + QB], ps[D:P, 1, :])

            q_proj(0, 2)

            # ---- v projection: [key, slot, dim] layout, odd slots; two key
            # blocks packed per PSUM tile (192-col halves).
            for i in range(0, JG, 2):
                blocks = [j for j in (i, i + 1) if j < JG]
                ps = pspool.tile([P, 2, QB], f32, name="psv", tag="ps")
                for s, j in enumerate(blocks):
                    for t in range(6):
                        nc.tensor.matmul(
                            ps[:, s, 0:192],
                            xTk[:, t, j * P : (j + 1) * P],
                            wv[:, t, :],
                            start=(t == 0),
                            stop=(t == 5),
                        )
                for s, j in enumerate(blocks):
                    nc.vector.tensor_copy(
                        v_sb[:, j, 1 : 2 * HPC : 2, :], ps[:, s, 0:192]
                    )

            # ---- attention per 512-query block.
            def attention(qb):
                qs = slice(qb * QB, (qb + 1) * QB)
                # heads 0,1: score pairs (h0 rows 0-63, h1 rows 64-127) per
                # key block; one exp covers both heads.
                po0 = popool.tile([P, QB], f32, name="po", tag="po")
                po1 = popool.tile([P, QB], f32, name="po", tag="po")
                for jg in range(JG):
                    ps = pspool.tile([P, 2, QB], f32, name="psc", tag="ps")
                    nc.tensor.matmul(
                        ps[:, 0, :],
                        kT01[0:D, jg * P : (jg + 1) * P],
                        qT01[0:D, qs],
                        start=True,
                        stop=True,
                    )
                    nc.tensor.matmul(
                        ps[:, 1, :],
                        kT01[D:P, jg * P : (jg + 1) * P],
                        qT01[D:P, qs],
                        start=True,
                        stop=True,
                    )
                    pt = ptpool.tile([P, 2, QB], bf16, name="pt", tag="pt")
                    nc.scalar.activation(pt[:], ps[:], Exp, scale=float(SCALE))
                    nc.tensor.matmul(
                        po0[:],
                        v_sb[:, jg, 0:2, :],
                        pt[:, 0, :],
                        start=(jg == 0),
                        stop=(jg == JG - 1),
                    )
                    nc.tensor.matmul(
                        po1[:],
                        v_sb[:, jg, 2:4, :],
                        pt[:, 1, :],
                        start=(jg == 0),
                        stop=(jg == JG - 1),
                    )
                rb0 = rbpool.tile([D, QB], f32, tag="rb")
                nc.vector.reciprocal_approx_fast(rb0[:], po0[0:D, :])
                nc.vector.tensor_mul(OT01[0:D, qb, :], po0[D:P, :], rb0[:])
                rb1 = rbpool.tile([D, QB], f32, tag="rb")
                nc.vector.reciprocal_approx_fast(rb1[:], po1[0:D, :])
                nc.vector.tensor_mul(OT01[D:P, qb, :], po1[D:P, :], rb1[:])

                # head 2: key-block pairs via the duplicated kT2/qT2 halves.
                po2 = popool.tile([P, QB], f32, name="po", tag="po")
                ntile = (JG + 1) // 2
                for p2 in range(ntile):
                    j0, j1 = 2 * p2, 2 * p2 + 1
                    hasb = j1 < JG
                    ps = pspool.tile([P, 2, QB], f32, name="psc", tag="ps")
                    nc.tensor.matmul(
                        ps[:, 0, :],
                        kT2d[0:D, j0 * P : (j0 + 1) * P],
                        qT2d[0:D, qs],
                        start=True,
                        stop=True,
                    )
                    if hasb:
                        nc.tensor.matmul(
                            ps[:, 1, :],
                            kT2d[D:P, j1 * P : (j1 + 1) * P],
                            qT2d[D:P, qs],
                            start=True,
                            stop=True,
                        )
                    pt = ptpool.tile([P, 2, QB], bf16, name="pt", tag="pt")
                    if hasb:
                        nc.scalar.activation(pt[:], ps[:], Exp, scale=float(SCALE))
                    else:
                        nc.scalar.activation(
                            pt[:, 0, :], ps[:, 0, :], Exp, scale=float(SCALE)
                        )
                    nc.tensor.matmul(
                        po2[:],
                        v_sb[:, j0, 4:6, :],
                        pt[:, 0, :],
                        start=(p2 == 0),
                        stop=(p2 == ntile - 1 and not hasb),
                    )
                    if hasb:
                        nc.tensor.matmul(
                            po2[:],
                            v_sb[:, j1, 4:6, :],
                            pt[:, 1, :],
                            start=False,
                            stop=(p2 == ntile - 1),
                        )
                rb2 = rbpool.tile([D, QB], f32, tag="rb")
                nc.vector.reciprocal_approx_fast(rb2[:], po2[0:D, :])
                nc.vector.tensor_mul(OT2d[0:D, qb, :], po2[D:P, :], rb2[:])
                dupq = nc.sync if qb % 2 == 0 else nc.gpsimd
                dupq.dma_start(OT2d[D:P, qb, :], OT2d[0:D, qb, :])

                # out-projection: contraction 192 = 128 (pT01, full rows)
                # + 64 (pT2, packed even/odd cg in row halves).
                ob = opool.tile([P, 6, QB], bf16, tag="ob")
                for j in range(3):
                    pp = pppool.tile([P, 2, QB], f32, name="pp", tag="pp")
                    nc.tensor.matmul(
                        pp[:, 0, :], pT01[:, 2 * j, :], OT01[:, qb, :],
                        start=True, stop=False,
                    )
                    nc.tensor.matmul(
                        pp[:, 1, :], pT01[:, 2 * j + 1, :], OT01[:, qb, :],
                        start=True, stop=False,
                    )
                    nc.tensor.matmul(
                        pp[:, 0, :], pT2[0:D, j, :], OT2d[0:D, qb, :],
                        start=False, stop=True,
                    )
                    nc.tensor.matmul(
                        pp[:, 1, :], pT2[D:P, j, :], OT2d[D:P, qb, :],
                        start=False, stop=True,
                    )
                    nc.vector.tensor_copy(ob[:, 2 * j : 2 * j + 2, :], pp[:])
                outq = nc.sync if qb % 2 == 0 else nc.gpsimd
                outq.dma_start(out_d[:, qb, :, :], ob[:])

            attention(0)
            q_proj(2, 3)
            attention(1)
            q_proj(3, 4)
            attention(2)
            attention(3)

    nc.finalize()
    return nc


def _prep_inputs(x, mask, qkv_w, proj_w):
    """Build the 8 per-core input maps. Returns (in_maps, KP)."""
    idx = [np.nonzero(mask[b] == 0.0)[0] for b in range(B)]
    nk = max(len(i) for i in idx)
    KP = max(P, int(math.ceil(nk / P)) * P)
    JG = KP // P

    per_batch = []
    for b in range(B):
        xTb = np.ascontiguousarray(x[b].T)  # [C, N] f32
        xT_in = xTb.reshape(6, P, N).transpose(1, 0, 2).astype(BF16)
        xk = np.zeros((C, KP), np.float32)
        xk[:, : len(idx[b])] = xTb[:, idx[b]]
        xTk_in = xk.reshape(6, P, KP).transpose(1, 0, 2).astype(BF16)
        kfv = np.zeros((KP,), np.float32)
        kfv[: len(idx[b])] = 1.0
        kfL_in = np.ascontiguousarray(
            np.broadcast_to(
                kfv[(JG - 1) * P :][:, None, None], (P, HPC, D)
            )
        ).astype(BF16)
        per_batch.append((xT_in, xTk_in, kfL_in))

    in_maps = []
    for c in range(NCORES):
        b, g = c // 4, c % 4
        h0 = HPC * g
        xT_in, xTk_in, kfL_in = per_batch[b]
        m = {"xT": xT_in, "xTk": xTk_in, "kfL": kfL_in}
        for name, off in (("wqT", 0), ("wkT", C), ("wvT", 2 * C)):
            w = qkv_w[off + h0 * D : off + (h0 + HPC) * D]  # [192, C]
            m[name] = (
                np.ascontiguousarray(w.T).reshape(6, P, 192).transpose(1, 0, 2).astype(BF16)
            )
        pw = proj_w[:, h0 * D : h0 * D + HPC * D]  # [768, 192]
        m["pT01"] = np.ascontiguousarray(pw[:, :P].T).reshape(P, 6, P).astype(BF16)
        pT2o = np.ascontiguousarray(pw[:, P:].T).reshape(D, 6, P)  # [64, 6, 128]
        pT2pk = np.empty((P, 3, P), np.float32)
        for j in range(3):
            pT2pk[0:D, j] = pT2o[:, 2 * j]
            pT2pk[D:P, j] = pT2o[:, 2 * j + 1]
        m["pT2pk"] = pT2pk.astype(BF16)
        in_maps.append(m)
    return in_maps, KP


_CACHE = {}


def _get_program(KP):
    if KP not in _CACHE:
        _CACHE[KP] = _build_program(KP)
    return _CACHE[KP]


def _gather_output(results, proj_b):
    out = np.empty((B, N, C), np.float32)
    for b in range(B):
        acc = None
        for c in range(4 * b, 4 * b + 4):
            a = results[c]["outT"]  # [128, NQB, 6, QB] bf16
            a = np.asarray(a, np.float32).transpose(2, 0, 1, 3).reshape(C, N)
            acc = a if acc is None else acc + a
        out[b] = acc.T + proj_b[None, :]
    return out


def kernel(x, mask, qkv_w, proj_w, proj_b, _want_results=False):
    from concourse.bass_utils import run_bass_kernel_spmd

    x = np.asarray(x, np.float32)
    mask = np.asarray(mask, np.float32)
    qkv_w = np.asarray(qkv_w, np.float32)
    proj_w = np.asarray(proj_w, np.float32)
    proj_b = np.asarray(proj_b, np.float32)

    in_maps, KP = _prep_inputs(x, mask, qkv_w, proj_w)
    nc = _get_program(KP)
    res = run_bass_kernel_spmd(nc, in_maps, list(range(NCORES)))

    out = _gather_output(res.results, proj_b)
    if _want_results:
        return out, res
    return out


# revision 5
# speedup vs baseline: 1.0428x; 1.0428x over previous
"""Masked multi-head attention kernel for 8 Trainium2 NeuronCores.

Strategy (v2 — PE-packing rework of the v1 baseline):
  - 24 (batch, head) pairs sharded as: core c -> batch c//4, heads [3*(c%4) .. 3*(c%4)+2].
  - Key-padding mask handled by HOST-side gather: only unmasked key positions are
    shipped/computed. Padded key slots get zeroed K columns (scores=0 -> exp=1)
    and a 0 in the indicator slot of V, so they contribute nothing.
  - Softmax without max-subtraction (scores ~ N(0,1); masked keys excluded).
  - Row-sum of exp folded into the AV matmul via an indicator slot on V.
  - PE tile packing (the big v2 win): all D=64-contraction matmuls run as
    concurrent PAIRS in the two 64-row halves of the PE array
    (tile_position auto-derived from partition bases):
      * scores: head0 (partitions 0-63) paired with head1 (64-127) per key
        block; head2 paired with itself via duplicated kT2/qT2 (dup'd by
        SBUF->SBUF DMA, free on the DMA engines).
      * out-proj: the K=64 tail (head2 dims) packed as column-group pairs
        (even cg rows 0-63, odd cg rows 64-127) against duplicated OT2.
      * qT2/kT2 projections: M=64 outputs packed 2 query-chunks per pass
        via column tiling.
  - One exp per score pair: [128, 2*512] ACTIVATE (amortizes ACT overhead,
    covers both heads of the pair).
  - ACT table preloaded by a dummy exp at t=0; all DMAs issued on the
    Sync + GpSimd queues so the Scalar engine does nothing but exp.
  - V indicator slots built on-device (gpsimd memset) + a tiny DMA for the
    padded tail block.
  - PSUM: 4 banks score pairs (x2), 2 banks AV accumulators, 2 banks proj.
  - bf16 matmul inputs, fp32 PSUM accumulation, bf16 output partials
    (host sums the 4 partials per batch in fp32, adds proj_b).
"""

import math

import numpy as np
import ml_dtypes

BF16 = ml_dtypes.bfloat16
B, N, C = 2, 2048, 768
H = 12
D = 64
HPC = 3          # heads per core
P = 128
QB = 512         # query block
NQB = N // QB
SCALE = D ** -0.5
NCORES = 8


def _chunks(total, size=QB):
    return [(o, min(size, total - o)) for o in range(0, total, size)]


def _build_program(KP: int):
    from concourse import bacc, mybir
    from concourse.tile import TileContext

    JG = KP // P
    f32 = mybir.dt.float32
    bf16 = mybir.dt.bfloat16
    Exp = mybir.ActivationFunctionType.Exp
    nc = bacc.Bacc(None, target_bir_lowering=False)

    xT_d = nc.declare_dram_parameter("xT", [P, 6, N], bf16, False)
    xTk_d = nc.declare_dram_parameter("xTk", [P, 6, KP], bf16, False)
    kfL_d = nc.declare_dram_parameter("kfL", [P, HPC, D], bf16, False)
    wq_d = nc.declare_dram_parameter("wqT", [P, 6, 192], bf16, False)
    wk_d = nc.declare_dram_parameter("wkT", [P, 6, 192], bf16, False)
    wv_d = nc.declare_dram_parameter("wvT", [P, 6, 192], bf16, False)
    pT01_d = nc.declare_dram_parameter("pT01", [P, 6, P], bf16, False)
    pT2_d = nc.declare_dram_parameter("pT2pk", [P, 3, P], bf16, False)
    out_d = nc.declare_dram_parameter("outT", [P, NQB, 6, QB], bf16, True)

    NHALF = N // 2

    with TileContext(nc) as tc:
        with (
            tc.tile_pool(name="const", bufs=1) as cpool,
            tc.tile_pool(name="work", bufs=1) as wpool,
            tc.tile_pool(name="pt", bufs=8) as ptpool,
            tc.tile_pool(name="rb", bufs=3) as rbpool,
            tc.tile_pool(name="outp", bufs=2) as opool,
            tc.tile_pool(name="ps", bufs=2, space="PSUM") as pspool,
            tc.tile_pool(name="po", bufs=2, space="PSUM") as popool,
            tc.tile_pool(name="pp", bufs=1, space="PSUM") as pppool,
        ):
            # ---- ACT exp-table preload: dummy exp with no data deps so the
            # ~2.7us ACT_TABLE_LOAD runs during the DMA phase.
            warm_i = cpool.tile([1, 8], f32)
            warm_o = cpool.tile([1, 8], bf16)
            nc.vector.memset(warm_i[:], 0.0)
            nc.scalar.activation(warm_o[:], warm_i[:], Exp)

            # ---- constant tiles + input DMAs.
            # Scalar queue: small weight tensors only (done before first exp).
            # Sync/GpSimd queues: bulk x / xk traffic. xTk first (gates k/v
            # proj), then xT in 512-query quarters so the qb0 q-projection
            # unblocks as early as possible.
            wq = cpool.tile([P, 6, 192], bf16)
            wk = cpool.tile([P, 6, 192], bf16)
            wv = cpool.tile([P, 6, 192], bf16)
            xT = cpool.tile([P, 6, N], bf16)
            xTk = cpool.tile([P, 6, KP], bf16)
            pT01 = cpool.tile([P, 6, P], bf16)
            pT2 = cpool.tile([P, 3, P], bf16)

            nc.scalar.dma_start(wk[:], wk_d[:])
            nc.scalar.dma_start(wq[:], wq_d[:])
            nc.scalar.dma_start(wv[:], wv_d[:])
            nc.scalar.dma_start(pT01[:], pT01_d[:])
            nc.scalar.dma_start(pT2[:], pT2_d[:])
            for t in range(6):
                q = nc.sync if t < 3 else nc.gpsimd
                q.dma_start(xTk[:, t : t + 1, :], xTk_d[:, t : t + 1, :])
            for qi, q in ((0, nc.sync), (1, nc.gpsimd), (2, nc.sync), (3, nc.gpsimd)):
                o = qi * QB
                for t in range(6):
                    q.dma_start(
                        xT[:, t : t + 1, o : o + QB], xT_d[:, t : t + 1, o : o + QB]
                    )

            # ---- work tiles
            qT01 = wpool.tile([P, N], bf16)
            qT2d = wpool.tile([P, N], bf16)   # head2 q, duplicated in halves
            kT01 = wpool.tile([P, KP], bf16)
            kT2d = wpool.tile([P, KP], bf16)  # head2 k, duplicated in halves
            v_sb = wpool.tile([P, JG, 2 * HPC, D], bf16)
            OT01 = wpool.tile([P, NQB, QB], bf16)
            OT2d = wpool.tile([P, NQB, QB], bf16)  # head2 O^T, duplicated

            # V indicator slots: 1.0 everywhere except the padded tail block,
            # which comes from a tiny DMA.
            if JG > 1:
                nc.gpsimd.memset(v_sb[:, 0 : JG - 1, 0 : 2 * HPC : 2, :], 1.0)
            nc.scalar.dma_start(v_sb[:, JG - 1, 0 : 2 * HPC : 2, :], kfL_d[:])

            kchunks = _chunks(KP)

            # ---- projection building blocks (each emits its own PSUM tile
            # allocs; emission order = scheduler priority).
            def k01_chunk(o, sz):
                ps = pspool.tile([P, 2, QB], f32, name="psq", tag="ps")
                for t in range(6):
                    nc.tensor.matmul(
                        ps[:, 0, 0:sz], wk[:, t, 0:P], xTk[:, t, o : o + sz],
                        start=(t == 0), stop=(t == 5),
                    )
                nc.vector.tensor_copy(kT01[:, o : o + sz], ps[:, 0, 0:sz])

            def k2_pair(pair):
                ps = pspool.tile([P, 2, QB], f32, name="psq2", tag="ps")
                (o0, sz0) = pair[0]
                for t in range(6):
                    nc.tensor.matmul(
                        ps[0:D, 0, 0:sz0], wk[:, t, P:192],
                        xTk[:, t, o0 : o0 + sz0],
                        start=(t == 0), stop=(t == 5),
                    )
                    if len(pair) == 2:
                        (o1, sz1) = pair[1]
                        nc.tensor.matmul(
                            ps[D:P, 1, 0:sz1], wk[:, t, P:192],
                            xTk[:, t, o1 : o1 + sz1],
                            start=(t == 0), stop=(t == 5),
                        )
                nc.vector.tensor_copy(kT2d[0:D, o0 : o0 + sz0], ps[0:D, 0, 0:sz0])
                if len(pair) == 2:
                    nc.vector.tensor_copy(
                        kT2d[0:D, pair[1][0] : pair[1][0] + pair[1][1]],
                        ps[D:P, 1, 0 : pair[1][1]],
                    )

            def q01_chunk(ci):
                o = ci * QB
                ps = pspool.tile([P, 2, QB], f32, name="psq", tag="ps")
                for t in range(6):
                    nc.tensor.matmul(
                        ps[:, 0, :], wq[:, t, 0:P], xT[:, t, o : o + QB],
                        start=(t == 0), stop=(t == 5),
                    )
                nc.vector.tensor_copy(qT01[:, o : o + QB], ps[:, 0, :])

            def q2_pair(c0, c1):
                # two query chunks packed in column halves, dup'd by DMA;
                # c1 may equal c0 (single chunk -> direct duplicate, no DMA)
                ps = pspool.tile([P, 2, QB], f32, name="psq2", tag="ps")
                o0, o1 = c0 * QB, c1 * QB
                for t in range(6):
                    nc.tensor.matmul(
                        ps[0:D, 0, :], wq[:, t, P:192], xT[:, t, o0 : o0 + QB],
                        start=(t == 0), stop=(t == 5),
                    )
                    nc.tensor.matmul(
                        ps[D:P, 1, :], wq[:, t, P:192], xT[:, t, o1 : o1 + QB],
                        start=(t == 0), stop=(t == 5),
                    )
                if c1 == c0:
                    nc.vector.tensor_copy(qT2d[0:D, o0 : o0 + QB], ps[0:D, 0, :])
                    nc.vector.tensor_copy(qT2d[D:P, o0 : o0 + QB], ps[D:P, 1, :])
                else:
                    nc.vector.tensor_copy(qT2d[0:D, o0 : o0 + QB], ps[0:D, 0, :])
                    nc.vector.tensor_copy(qT2d[0:D, o1 : o1 + QB], ps[D:P, 1, :])
                    nc.sync.dma_start(
                        qT2d[D:P, o0 : o1 + QB], qT2d[0:D, o0 : o1 + QB]
                    )

            def v_pair(i):
                blocks = [j for j in (i, i + 1) if j < JG]
                ps = pspool.tile([P, 2, QB], f32, name="psv", tag="ps")
                for s, j in enumerate(blocks):
                    for t in range(6):
                        nc.tensor.matmul(
                            ps[:, s, 0:192],
                            xTk[:, t, j * P : (j + 1) * P],
                            wv[:, t, :],
                            start=(t == 0), stop=(t == 5),
                        )
                for s, j in enumerate(blocks):
                    nc.vector.tensor_copy(
                        v_sb[:, j, 1 : 2 * HPC : 2, :], ps[:, s, 0:192]
                    )

            # ---- attention phases. scores_phase emits score pairs + exps
            # only (ACT critical chain); av_phase the AV matmuls + softmax
            # normalization; proj_phase the output projection. Fillers are
            # emitted between phases so they soak PE slack without ever
            # outranking the exp chain.
            def scores_h01(qb, jgs):
                qs = slice(qb * QB, (qb + 1) * QB)
                pts = []
                for jg in jgs:
                    ps = pspool.tile([P, 2, QB], f32, name="psc", tag="ps")
                    nc.tensor.matmul(
                        ps[:, 0, :], kT01[0:D, jg * P : (jg + 1) * P],
                        qT01[0:D, qs], start=True, stop=True,
                    )
                    nc.tensor.matmul(
                        ps[:, 1, :], kT01[D:P, jg * P : (jg + 1) * P],
                        qT01[D:P, qs], start=True, stop=True,
                    )
                    pt = ptpool.tile([P, 2, QB], bf16, name="pt", tag="pt")
                    nc.scalar.activation(pt[:], ps[:], Exp, scale=float(SCALE))
                    pts.append(pt)
                return pts

            def scores_h2(qb):
                qs = slice(qb * QB, (qb + 1) * QB)
                pts = []
                ntile = (JG + 1) // 2
                for p2 in range(ntile):
                    j0, j1 = 2 * p2, 2 * p2 + 1
                    hasb = j1 < JG
                    ps = pspool.tile([P, 2, QB], f32, name="psc", tag="ps")
                    nc.tensor.matmul(
                        ps[:, 0, :], kT2d[0:D, j0 * P : (j0 + 1) * P],
                        qT2d[0:D, qs], start=True, stop=True,
                    )
                    if hasb:
                        nc.tensor.matmul(
                            ps[:, 1, :], kT2d[D:P, j1 * P : (j1 + 1) * P],
                            qT2d[D:P, qs], start=True, stop=True,
                        )
                    pt = ptpool.tile([P, 2, QB], bf16, name="pt", tag="pt")
                    if hasb:
                        nc.scalar.activation(pt[:], ps[:], Exp, scale=float(SCALE))
                    else:
                        nc.scalar.activation(
                            pt[:, 0, :], ps[:, 0, :], Exp, scale=float(SCALE)
                        )
                    pts.append(pt)
                return pts

            def av_h01(qb, pts):
                po0 = popool.tile([P, QB], f32, name="po", tag="po")
                po1 = popool.tile([P, QB], f32, name="po", tag="po")
                for jg in range(JG):
                    nc.tensor.matmul(
                        po0[:], v_sb[:, jg, 0:2, :], pts[jg][:, 0, :],
                        start=(jg == 0), stop=(jg == JG - 1),
                    )
                    nc.tensor.matmul(
                        po1[:], v_sb[:, jg, 2:4, :], pts[jg][:, 1, :],
                        start=(jg == 0), stop=(jg == JG - 1),
                    )
                rb0 = rbpool.tile([D, QB], f32, tag="rb")
                nc.vector.reciprocal_approx_fast(rb0[:], po0[0:D, :])
                nc.vector.tensor_mul(OT01[0:D, qb, :], po0[D:P, :], rb0[:])
                rb1 = rbpool.tile([D, QB], f32, tag="rb")
                nc.vector.reciprocal_approx_fast(rb1[:], po1[0:D, :])
                nc.vector.tensor_mul(OT01[D:P, qb, :], po1[D:P, :], rb1[:])

            def av_h2(qb, pts):
                po2 = popool.tile([P, QB], f32, name="po", tag="po")
                ntile = (JG + 1) // 2
                for p2 in range(ntile):
                    j0, j1 = 2 * p2, 2 * p2 + 1
                    hasb = j1 < JG
                    nc.tensor.matmul(
                        po2[:], v_sb[:, j0, 4:6, :], pts[p2][:, 0, :],
                        start=(p2 == 0), stop=(p2 == ntile - 1 and not hasb),
                    )
                    if hasb:
                        nc.tensor.matmul(
                            po2[:], v_sb[:, j1, 4:6, :], pts[p2][:, 1, :],
                            start=False, stop=(p2 == ntile - 1),
                        )
                rb2 = rbpool.tile([D, QB], f32, tag="rb")
                nc.vector.reciprocal_approx_fast(rb2[:], po2[0:D, :])
                nc.vector.tensor_mul(OT2d[0:D, qb, :], po2[D:P, :], rb2[:])
                dupq = nc.sync if qb % 2 == 0 else nc.gpsimd
                dupq.dma_start(OT2d[D:P, qb, :], OT2d[0:D, qb, :])

            def proj_phase(qb):
                ob = opool.tile([P, 6, QB], bf16, tag="ob")
                for j in range(3):
                    pp = pppool.tile([P, 2, QB], f32, name="pp", tag="pp")
                    nc.tensor.matmul(
                        pp[:, 0, :], pT01[:, 2 * j, :], OT01[:, qb, :],
                        start=True, stop=False,
                    )
                    nc.tensor.matmul(
                        pp[:, 1, :], pT01[:, 2 * j + 1, :], OT01[:, qb, :],
                        start=True, stop=False,
                    )
                    nc.tensor.matmul(
                        pp[:, 0, :], pT2[0:D, j, :], OT2d[0:D, qb, :],
                        start=False, stop=True,
                    )
                    nc.tensor.matmul(
                        pp[:, 1, :], pT2[D:P, j, :], OT2d[D:P, qb, :],
                        start=False, stop=True,
                    )
                    nc.vector.tensor_copy(ob[:, 2 * j : 2 * j + 2, :], pp[:])
                outq = nc.sync if qb % 2 == 0 else nc.gpsimd
                outq.dma_start(out_d[:, qb, :, :], ob[:])

            # ---- emission schedule.
            # qb0 is hand-interleaved: the critical chain (kT01 c0, qT01 c0,
            # first scores+exp) goes first; everything else slots into the
            # PE slack of the qb0 exp stream.
            njg_c0 = min(JG, 4)
            k01_chunk(*kchunks[0])
            q01_chunk(0)
            pts0 = scores_h01(0, range(njg_c0))
            for o, sz in kchunks[1:]:
                k01_chunk(o, sz)
            v_pair(0)
            v_pair(2)
            if JG > njg_c0:
                pts0 += scores_h01(0, range(njg_c0, JG))
            for i in range(0, len(kchunks), 2):
                k2_pair(kchunks[i : i + 2])
            nc.sync.dma_start(kT2d[D:P, :], kT2d[0:D, :])
            q2_pair(0, 1)
            v_pair(4)
            v_pair(6)
            if JG > 8:
                for i in range(8, JG, 2):
                    v_pair(i)
            pts0h2 = scores_h2(0)
            av_h01(0, pts0)
            av_h2(0, pts0h2)

            q01_chunk(1)
            pts1 = scores_h01(1, range(JG))
            pts1h2 = scores_h2(1)
            proj_phase(0)
            q01_chunk(2)
            q2_pair(2, 2)
            av_h01(1, pts1)
            av_h2(1, pts1h2)

            pts2 = scores_h01(2, range(JG))
            pts2h2 = scores_h2(2)
            proj_phase(1)
            q01_chunk(3)
            q2_pair(3, 3)
            av_h01(2, pts2)
            av_h2(2, pts2h2)

            pts3 = scores_h01(3, range(JG))
            pts3h2 = scores_h2(3)
            proj_phase(2)
            av_h01(3, pts3)
            av_h2(3, pts3h2)
            proj_phase(3)

    nc.finalize()
    return nc


def _prep_inputs(x, mask, qkv_w, proj_w):
    """Build the 8 per-core input maps. Returns (in_maps, KP)."""
    idx = [np.nonzero(mask[b] == 0.0)[0] for b in range(B)]
    nk = max(len(i) for i in idx)
    KP = max(P, int(math.ceil(nk / P)) * P)
    JG = KP // P

    per_batch = []
    for b in range(B):
        xTb = np.ascontiguousarray(x[b].T)  # [C, N] f32
        xT_in = xTb.reshape(6, P, N).transpose(1, 0, 2).astype(BF16)
        xk = np.zeros((C, KP), np.float32)
        xk[:, : len(idx[b])] = xTb[:, idx[b]]
        xTk_in = xk.reshape(6, P, KP).transpose(1, 0, 2).astype(BF16)
        kfv = np.zeros((KP,), np.float32)
        kfv[: len(idx[b])] = 1.0
        kfL_in = np.ascontiguousarray(
            np.broadcast_to(
                kfv[(JG - 1) * P :][:, None, None], (P, HPC, D)
            )
        ).astype(BF16)
        per_batch.append((xT_in, xTk_in, kfL_in))

    in_maps = []
    for c in range(NCORES):
        b, g = c // 4, c % 4
        h0 = HPC * g
        xT_in, xTk_in, kfL_in = per_batch[b]
        m = {"xT": xT_in, "xTk": xTk_in, "kfL": kfL_in}
        for name, off in (("wqT", 0), ("wkT", C), ("wvT", 2 * C)):
            w = qkv_w[off + h0 * D : off + (h0 + HPC) * D]  # [192, C]
            m[name] = (
                np.ascontiguousarray(w.T).reshape(6, P, 192).transpose(1, 0, 2).astype(BF16)
            )
        pw = proj_w[:, h0 * D : h0 * D + HPC * D]  # [768, 192]
        m["pT01"] = np.ascontiguousarray(pw[:, :P].T).reshape(P, 6, P).astype(BF16)
        pT2o = np.ascontiguousarray(pw[:, P:].T).reshape(D, 6, P)  # [64, 6, 128]
        pT2pk = np.empty((P, 3, P), np.float32)
        for j in range(3):
            pT2pk[0:D, j] = pT2o[:, 2 * j]
            pT2pk[D:P, j] = pT2o[:, 2 * j + 1]
        m["pT2pk"] = pT2pk.astype(BF16)
        in_maps.append(m)
    return in_maps, KP


_CACHE = {}


def _get_program(KP):
    if KP not in _CACHE:
        _CACHE[KP] = _build_program(KP)
    return _CACHE[KP]


def _gather_output(results, proj_b):
    out = np.empty((B, N, C), np.float32)
    for b in range(B):
        acc = None
        for c in range(4 * b, 4 * b + 4):
            a = results[c]["outT"]  # [128, NQB, 6, QB] bf16
            a = np.asarray(a, np.float32).transpose(2, 0, 1, 3).reshape(C, N)
            acc = a if acc is None else acc + a
        out[b] = acc.T + proj_b[None, :]
    return out


def kernel(x, mask, qkv_w, proj_w, proj_b, _want_results=False):
    from concourse.bass_utils import run_bass_kernel_spmd

    x = np.asarray(x, np.float32)
    mask = np.asarray(mask, np.float32)
    qkv_w = np.asarray(qkv_w, np.float32)
    proj_w = np.asarray(proj_w, np.float32)
    proj_b = np.asarray(proj_b, np.float32)

    in_maps, KP = _prep_inputs(x, mask, qkv_w, proj_w)
    nc = _get_program(KP)
    res = run_bass_kernel_spmd(nc, in_maps, list(range(NCORES)))

    out = _gather_output(res.results, proj_b)
    if _want_results:
        return out, res
    return out
